# revision 5
# baseline (speedup 1.0000x reference)
"""Causal dilated conv1d (K=3, dilation=2, N=128 channels) on Trainium2.

out[b,t,i] = sum_{j,k} x[b, t-2k, j] * weight[i,j,k] + bias[i]

Strategy (8-core SPMD, pure data parallel over batch):
  - each core handles 4 of the 32 batch rows; weight/bias replicated
  - x is pre-transposed on the HOST to [B, N, T] fp16, so the device
    does zero PE transposes: per batch row a [128(j), T+4] strip (4-col
    zero halo on the left turns the dilated taps into plain column
    offsets) is DMA'd straight in, and the conv is just 3 accumulated
    fp16 matmuls per 512-wide window: out_T[i,t] = sum_k w_k^T @ xT[:,t-2k].
  - bias is added during the PSUM->SBUF copy (fp16 out), alternating
    between the Scalar and Vector engines.
  - DMA queues are descriptor-rate limited (~130ns per 4KB descriptor,
    one dma_start lands on one queue), so every bulk transfer is split
    across partition ranges to engage several queues in parallel, and
    all input DMAs are issued up-front (4 resident strips) so they are
    never queued behind output doorbells.
  - output is written in [B, N, T] fp16 layout (contiguous per-partition
    runs) and un-transposed / upcast to fp32 on the host.
"""

import threading

import numpy as np

import concourse.bass as bass  # noqa: F401  (bass types used via bacc/tile)
import concourse.mybir as mybir
import concourse.tile as tile
from concourse import bacc
from concourse.bass_utils import run_bass_kernel_spmd

P = 128
KTAPS = 3
DIL = 2
HALO = (KTAPS - 1) * DIL  # 4
NCORES = 8
B_FULL, T_FULL = 32, 8192
B_CORE = B_FULL // NCORES  # 4

FP32 = mybir.dt.float32
FP16 = mybir.dt.float16


def build(Bc=B_CORE, T=T_FULL, chunk=2048):
    """Build the per-core Bass module. Same NEFF runs SPMD on all 8 cores."""
    nc = bacc.Bacc(
        "TRN2",
        target_bir_lowering=False,
        debug=False,
        enable_asserts=False,
        num_devices=NCORES,
    )
    x_d = nc.dram_tensor("x", [Bc, P, T], FP16, kind="ExternalInput")
    w_d = nc.dram_tensor("w", [P, KTAPS * P], FP16, kind="ExternalInput")
    b_d = nc.dram_tensor("b", [P, 1], FP32, kind="ExternalInput")
    o_d = nc.dram_tensor("o", [Bc, P, T], FP16, kind="ExternalOutput")

    x_ap, o_ap = x_d.ap(), o_d.ap()
    n_chunks = T // chunk
    SW = 512  # tap-matmul moving width (1 PSUM bank of fp32)
    S = chunk // SW  # strips per chunk

    with tile.TileContext(nc) as tc:
        with (
            tc.tile_pool(name="const", bufs=1) as cp,
            tc.tile_pool(name="strip", bufs=1) as sp,
            tc.tile_pool(name="ob", bufs=1) as op,
            tc.tile_pool(name="pacc", bufs=6, space="PSUM") as paccp,
        ):
            w_sb = cp.tile([P, KTAPS * P], FP16)
            nc.sync.dma_start(w_sb[:], w_d.ap())
            bias_sb = cp.tile([P, 1], FP32)
            nc.sync.dma_start(bias_sb[:], b_d.ap())

            # front-load ALL input DMAs: strips for every batch row are
            # resident simultaneously, so the input stream never queues
            # behind output-DMA doorbells on the sync engine.  Each column
            # piece is split across 4 partition ranges -> 4 parallel queues.
            PS = 32
            strips = []
            for b in range(Bc):
                strip = sp.tile([P, T + HALO], FP16, tag=f"strip{b}")
                nc.vector.memset(strip[:, 0:HALO], 0.0)
                if b == 0:
                    # ramp-up: fine leading pieces so the PE starts ASAP
                    pieces = [(0, 512), (512, 2048), (2048, 4096), (4096, 8192)]
                else:
                    pieces = [(0, 4096), (4096, 8192)]
                for c0, c1 in pieces:
                    for p in range(0, P, PS):
                        nc.sync.dma_start(
                            strip[p : p + PS, HALO + c0 : HALO + c1],
                            x_ap[b, p : p + PS, c0:c1],
                        )
                strips.append(strip)

            # PSUM->SBUF copy-with-bias engines, rotated per strip
            copy_engines = [
                lambda o, i: nc.scalar.add(o, i, bias_sb),
                lambda o, i: nc.vector.tensor_scalar_add(o, i, bias_sb),
            ]
            OPS = 16  # out-DMA partition split -> 8 parallel queues per ring
            cnt = 0
            for b in range(Bc):
                strip = strips[b]
                ob = op.tile([P, T], FP16, tag=f"ob{b}")
                for ci in range(n_chunks):
                    t0 = ci * chunk
                    for s in range(S):
                        st = t0 + s * SW
                        pacc = paccp.tile([P, SW], FP32, tag="pacc")
                        for k in range(KTAPS):
                            off = HALO + st - DIL * k
                            nc.tensor.matmul(
                                pacc[:],
                                w_sb[:, k * P : (k + 1) * P],
                                strip[:, off : off + SW],
                                start=(k == 0),
                                stop=(k == KTAPS - 1),
                            )
                        copy_engines[cnt % len(copy_engines)](
                            ob[:, st : st + SW], pacc[:]
                        )
                        cnt += 1
                    for p in range(0, P, OPS):
                        nc.sync.dma_start(
                            o_ap[b, p : p + OPS, t0 : t0 + chunk],
                            ob[p : p + OPS, t0 : t0 + chunk],
                        )
    nc.compile()
    return nc


_cache = {}
_lock = threading.Lock()


def _get_nc():
    with _lock:
        if "nc" not in _cache:
            _cache["nc"] = build()
        return _cache["nc"]


def prep_inputs(x, weight, bias):
    # x -> [B, N, T] fp16 (host transpose; device then needs no PE transposes)
    xt = np.swapaxes(np.asarray(x, np.float32), 1, 2).astype(np.float16)
    # w_all[j, k*128 + i] = weight[i, j, k]
    w_all = np.ascontiguousarray(
        np.transpose(np.asarray(weight, np.float32), (1, 2, 0)).reshape(P, KTAPS * P)
    ).astype(np.float16)
    b2 = np.ascontiguousarray(np.asarray(bias, np.float32).reshape(P, 1))
    return np.ascontiguousarray(xt), w_all, b2


def kernel(x, weight, bias, _trace=False):
    xt, w_all, b2 = prep_inputs(x, weight, bias)
    nc = _get_nc()
    in_maps = [
        {"x": xt[c * B_CORE : (c + 1) * B_CORE], "w": w_all, "b": b2}
        for c in range(NCORES)
    ]
    res = run_bass_kernel_spmd(nc, in_maps, core_ids=list(range(NCORES)), trace=_trace)
    ot = np.concatenate([r["o"] for r in res.results], axis=0)  # [B, N, T] fp16
    out = np.swapaxes(ot, 1, 2).astype(np.float32)
    if _trace:
        kernel.last_results = res
    return np.ascontiguousarray(out)


# revision 6
# speedup vs baseline: 2.1249x; 2.1249x over previous
"""Causal dilated conv1d (K=3, dilation=2, N=128 channels) on Trainium2.

out[b,t,i] = sum_{j,k} x[b, t-2k, j] * weight[i,j,k] + bias[i]

Strategy (8-core SPMD, pure data parallel over batch):
  - each core handles 4 of the 32 batch rows; weight/bias replicated
  - x is pre-transposed on the HOST to [B, N, T] fp16, so the device
    does zero PE transposes: per batch row a [128(j), T+4] strip (4-col
    zero halo on the left turns the dilated taps into plain column
    offsets) is DMA'd straight in, and the conv is just 3 accumulated
    fp16 matmuls per 512-wide window: out_T[i,t] = sum_k w_k^T @ xT[:,t-2k].
  - bias is added during the PSUM->SBUF copy (fp16 out), alternating
    between the Scalar and Vector engines.
  - DMA rules (05-dma-engines.md): one dma_start already fans out over
    all 16 SDMA engines; partition-split transfers serialize on the same
    rings, so every transfer is full-128-partition and large (~0.5-1MB),
    with a small leading piece so the PE starts early.  All input DMAs
    are issued up-front; outputs accumulate into per-batch-row SBUF
    buffers so no engine ever waits on an output DMA (no WAR chains).
  - output is written in [B, N, T] fp16 layout (contiguous per-partition
    runs) and un-transposed / upcast to fp32 on the host.
"""

import threading

import numpy as np

import concourse.bass as bass  # noqa: F401  (bass types used via bacc/tile)
import concourse.mybir as mybir
import concourse.tile as tile
from concourse import bacc
from concourse.bass_utils import run_bass_kernel_spmd

P = 128
KTAPS = 3
DIL = 2
HALO = (KTAPS - 1) * DIL  # 4
NCORES = 8
B_FULL, T_FULL = 32, 8192
B_CORE = B_FULL // NCORES  # 4

FP32 = mybir.dt.float32
FP16 = mybir.dt.float16


def build(Bc=B_CORE, T=T_FULL, chunk=2048):
    """Build the per-core Bass module. Same NEFF runs SPMD on all 8 cores."""
    nc = bacc.Bacc(
        "TRN2",
        target_bir_lowering=False,
        debug=False,
        enable_asserts=False,
        num_devices=NCORES,
    )
    x_d = nc.dram_tensor("x", [Bc, P, T], FP16, kind="ExternalInput")
    w_d = nc.dram_tensor("w", [P, KTAPS * P], FP16, kind="ExternalInput")
    b_d = nc.dram_tensor("b", [P, 1], FP32, kind="ExternalInput")
    o_d = nc.dram_tensor("o", [Bc, P, T], FP16, kind="ExternalOutput")

    x_ap, o_ap = x_d.ap(), o_d.ap()
    n_chunks = T // chunk
    SW = 512  # tap-matmul moving width (1 PSUM bank of fp32)
    S = chunk // SW  # strips per chunk

    with tile.TileContext(nc) as tc:
        with (
            tc.tile_pool(name="const", bufs=1) as cp,
            tc.tile_pool(name="strip", bufs=1) as sp,
            tc.tile_pool(name="ob", bufs=1) as op,
            tc.tile_pool(name="pacc", bufs=6, space="PSUM") as paccp,
        ):
            w_sb = cp.tile([P, KTAPS * P], FP16)
            nc.sync.dma_start(w_sb[:], w_d.ap())
            bias_sb = cp.tile([P, 1], FP32)
            nc.sync.dma_start(bias_sb[:], b_d.ap())

            # front-load ALL input DMAs (always full 128 partitions):
            # a short leading piece for b0 so the PE starts ASAP, then
            # large ~0.5-1MB transfers for DMA efficiency.
            strips = []
            for b in range(Bc):
                strip = sp.tile([P, T + HALO], FP16, tag=f"strip{b}")
                nc.vector.memset(strip[:, 0:HALO], 0.0)
                if b == 0:
                    pieces = [(0, 512), (512, 2048), (2048, 4096), (4096, 8192)]
                else:
                    pieces = [(0, 4096), (4096, 8192)]
                for c0, c1 in pieces:
                    nc.sync.dma_start(
                        strip[:, HALO + c0 : HALO + c1], x_ap[b, :, c0:c1]
                    )
                strips.append(strip)

            # PSUM->SBUF copy-with-bias engines, rotated per strip
            copy_engines = [
                lambda o, i: nc.scalar.add(o, i, bias_sb),
                lambda o, i: nc.vector.tensor_scalar_add(o, i, bias_sb),
            ]
            cnt = 0
            for b in range(Bc):
                strip = strips[b]
                ob = op.tile([P, T], FP16, tag=f"ob{b}")
                for ci in range(n_chunks):
                    t0 = ci * chunk
                    for s in range(S):
                        st = t0 + s * SW
                        pacc = paccp.tile([P, SW], FP32, tag="pacc")
                        for k in range(KTAPS):
                            off = HALO + st - DIL * k
                            nc.tensor.matmul(
                                pacc[:],
                                w_sb[:, k * P : (k + 1) * P],
                                strip[:, off : off + SW],
                                start=(k == 0),
                                stop=(k == KTAPS - 1),
                            )
                        copy_engines[cnt % len(copy_engines)](
                            ob[:, st : st + SW], pacc[:]
                        )
                        cnt += 1
                    nc.sync.dma_start(
                        o_ap[b, :, t0 : t0 + chunk], ob[:, t0 : t0 + chunk]
                    )
    nc.compile()
    return nc


_cache = {}
_lock = threading.Lock()


def _get_nc():
    with _lock:
        if "nc" not in _cache:
            _cache["nc"] = build()
        return _cache["nc"]


def prep_inputs(x, weight, bias):
    # x -> [B, N, T] fp16 (host transpose; device then needs no PE transposes)
    xt = np.swapaxes(np.asarray(x, np.float32), 1, 2).astype(np.float16)
    # w_all[j, k*128 + i] = weight[i, j, k]
    w_all = np.ascontiguousarray(
        np.transpose(np.asarray(weight, np.float32), (1, 2, 0)).reshape(P, KTAPS * P)
    ).astype(np.float16)
    b2 = np.ascontiguousarray(np.asarray(bias, np.float32).reshape(P, 1))
    return np.ascontiguousarray(xt), w_all, b2


def kernel(x, weight, bias, _trace=False):
    xt, w_all, b2 = prep_inputs(x, weight, bias)
    nc = _get_nc()
    in_maps = [
        {"x": xt[c * B_CORE : (c + 1) * B_CORE], "w": w_all, "b": b2}
        for c in range(NCORES)
    ]
    res = run_bass_kernel_spmd(nc, in_maps, core_ids=list(range(NCORES)), trace=_trace)
    ot = np.concatenate([r["o"] for r in res.results], axis=0)  # [B, N, T] fp16
    out = np.swapaxes(ot, 1, 2).astype(np.float32)
    if _trace:
        kernel.last_results = res
    return np.ascontiguousarray(out)
